# revision 8
# baseline (speedup 1.0000x reference)
"""2D DCT-II (4096x4096, fp32) on 8 TRN2 NeuronCores, bf16 matmuls.

out = C0 @ x @ C1^T with C0 = C1 = C, C[k, i] = cos(pi*(2i+1)*k/(2N)).

Fast-DCT folding via the basis reflection symmetries, applied on BOTH
axes. Column (v) axis: level-1 on the host (xa/xb halves), levels 2+3
on-device via DVE butterflies (unchanged from the fp32r baseline).
Row (u) axis: ALL levels on the host -- every core gets a balanced mix
of u-classes so no core is stuck with unfoldable work:

  core c (all 8):  256 odd-u rows   u = 512c+1..512c+511 step 2
                   -> contraction 2048 over xO (host level-1 diff)
  cores 0-3:       256 u%4==2 rows  u = 1024c+2 step 4
                   -> contraction 1024 over E2d = xE[:Q]-xE[Q:][::-1]
  cores 4-7 (e):   128 u%8==0 rows (u=1024e step 8)   over E3s [512]
                   128 u%8==4 rows (u=1024e+4 step 8) over E3d [512]

The folded-class contraction is UNIFIED across core types: xf [1024,*]
is E2d for %4-cores and [E3s; E3d] stacked for %8-cores, whose basis
c0f [1024, 256] is block-diagonal (zeros kill the cross terms). One
SPMD program, zero branching; zero extra DVE work (host row-folds are
free and replace the raw xE stream byte-for-byte).

Device pipeline per core (bf16 operands, fp32 PSUM):
  stage 1, per (half h, j'-group g): one PSUM bank per 128-j'-block;
    odd-u:   16 kt x moving-256 into cols 0:256   (lhsT = xo tiles)
    folded:   8 kt x moving-256 into cols 256:512 (lhsT = xf tiles)
    -> 768 matmuls total, intermediates land [j', m] in SBUF = the
    stationary layout stage 2 needs. m-packing = [odd 256 | folded 256].
  butterflies: t2e/t2o = TE'[:Q] -+ TE'[Q:];  t3e/t3o = t2e[:E] -+ t2e[E:]
  stage 2 (unchanged): v%8==0/4 (512-deep), v%4==2 (1024), v odd (2048)
    = 352 matmuls, all moving-512.
  Output leaves section-packed [v80|v84|v2|vodd] x [odd|folded] rows;
  the host scatters (pure numpy slicing).

PE work: 768*107ns + 352*213ns ~ 157us + warmup; HBM ~37 MB/core.
"""

import math

import numpy as np

import concourse.mybir as mybir
import concourse.tile as tile
from concourse import bacc
from concourse.bass_utils import run_bass_kernel_spmd

N = 4096
H = N // 2  # 2048: level-1 folded contraction
Q = N // 4  # 1024: level-2 folded contraction
E = N // 8  # 512:  level-3 folded contraction
P = 128
HT = H // P  # 16
QT = Q // P  # 8
ET = E // P  # 4
NCORES = 8
RB = 512  # output rows per core
RO = 256  # odd-u rows per core (psum cols 0:RO)
G = 512  # column-group / matmul moving width (stage 2)
KQ = 4  # k-tiles per streaming DMA

f32 = mybir.dt.float32
bf16 = mybir.dt.bfloat16

_CACHE = {}


def _build():
    nc = bacc.Bacc("TRN2", target_bir_lowering=False, debug=False)
    xoa_d = nc.dram_tensor("xoa", [H, H], bf16, kind="ExternalInput")
    xob_d = nc.dram_tensor("xob", [H, H], bf16, kind="ExternalInput")
    xfa_d = nc.dram_tensor("xfa", [Q, H], bf16, kind="ExternalInput")
    xfb_d = nc.dram_tensor("xfb", [Q, H], bf16, kind="ExternalInput")
    c0o_d = nc.dram_tensor("c0o", [H, RO], bf16, kind="ExternalInput")
    c0f_d = nc.dram_tensor("c0f", [Q, RO], bf16, kind="ExternalInput")
    c1v8_d = nc.dram_tensor("c1v8", [E, Q], bf16, kind="ExternalInput")
    c1v2_d = nc.dram_tensor("c1v2", [Q, Q], bf16, kind="ExternalInput")
    c1vo_d = nc.dram_tensor("c1vo", [H, H], bf16, kind="ExternalInput")
    out_d = nc.dram_tensor("out", [RB, N], bf16, kind="ExternalOutput")

    state = {"ggc": 0}

    with tile.TileContext(nc) as tc:
        with (
            tc.tile_pool(name="persist", bufs=1) as persist,
            tc.tile_pool(name="xin", bufs=6) as xin,
            tc.tile_pool(name="fin", bufs=3) as fin,
            tc.tile_pool(name="cin", bufs=6) as cin,
            tc.tile_pool(name="osb", bufs=3) as osb,
            tc.tile_pool(name="ps", bufs=1, space="PSUM") as ps,
        ):
            c0o_sb = persist.tile([P, HT, RO], bf16, tag="c0o", name="c0o_sb")
            c0f_sb = persist.tile([P, QT, RO], bf16, tag="c0f", name="c0f_sb")
            # intermediates [j', m] as [128, 16, 512]; m = [odd | folded]
            t_sb = [
                persist.tile([P, HT, RB], bf16, tag=f"t{h}", name=f"t{h}_sb")
                for h in range(2)
            ]

            def banks(n=4):
                g = state["ggc"]
                state["ggc"] += 1
                return [
                    ps.tile(
                        [P, G], f32, tag=f"ps{(g % 2) * 4 + i}",
                        name=f"ps{(g % 2) * 4 + i}",
                    )
                    for i in range(n)
                ]

            def drain(bk, mb, dst):
                # alternate DVE/ACT so section-end drains parallelize
                if mb % 2 == 0:
                    nc.vector.tensor_copy(dst, bk[:])
                else:
                    nc.scalar.copy(dst, bk[:])

            # PE warm-up: HAM clock gate needs ~3.4us of sustained matmul
            # activity; chew zeros while the first operands stream in.
            junk = persist.tile([P, P], f32, tag="junk", name="junk")
            nc.gpsimd.memset(junk[:], 0)
            jps = ps.tile([P, P], f32, tag="ps7", name="jps")
            for _ in range(20):
                nc.tensor.matmul(jps[:], junk[:], junk[:], start=True, stop=True)

            # ---- stage 1 ----
            # T(a|b)[j', m]: odd-u (16 kt over xo) + folded (8 kt over xf)
            for h in range(2):
                xo_src = xoa_d if h == 0 else xob_d
                xf_src = xfa_d if h == 0 else xfb_d
                for g in range(4):  # j'-column groups of 512
                    bk = banks()
                    # odd-u: contraction 2048, psum cols 0:256
                    for kq in range(HT // KQ):
                        if h == 0 and g == 0 and kq == 0:
                            # fine-grained first chunk: c0o k-tiles lead
                            # their x partners on the same (sync) queue so
                            # the first matmul starts after ~192 KB
                            for ko in range(KQ):
                                nc.sync.dma_start(
                                    c0o_sb[:, ko, :],
                                    c0o_d[ko * P:(ko + 1) * P, :],
                                )
                                if ko == 0:
                                    xt = xin.tile(
                                        [P, KQ, G], bf16, tag="xt", name="xt"
                                    )
                                nc.sync.dma_start(
                                    xt[:, ko, :],
                                    xo_src[ko * P:(ko + 1) * P, 0:G],
                                )
                            nc.sync.dma_start(
                                c0o_sb[:, KQ:, :],
                                c0o_d[KQ * P:, :].rearrange(
                                    "(o p) m -> p o m", p=P
                                ),
                            )
                        else:
                            xt = xin.tile([P, KQ, G], bf16, tag="xt", name="xt")
                            nc.sync.dma_start(
                                xt[:],
                                xo_src[
                                    kq * KQ * P:(kq + 1) * KQ * P,
                                    g * G:(g + 1) * G,
                                ].rearrange("(o p) n -> p o n", p=P),
                            )
                        for ko in range(KQ):
                            it = kq * KQ + ko
                            for jb in range(4):
                                nc.tensor.matmul(
                                    bk[jb][:, 0:RO],
                                    xt[:, ko, jb * P:(jb + 1) * P],
                                    c0o_sb[:, it, :],
                                    start=(it == 0),
                                    stop=False,
                                )
                        if h == 0 and g == 0 and kq == 0:
                            # c0f for the folded class, early on gpsimd
                            nc.gpsimd.dma_start(
                                c0f_sb[:],
                                c0f_d.rearrange("(o p) m -> p o m", p=P),
                            )
                        if h == 0 and g < 2 and (g > 0 or kq >= 2):
                            # idle-bank warm fillers across the early
                            # HBM-starved chunk boundaries
                            ftag = "ps4" if g == 0 else "ps0"
                            fps = ps.tile([P, P], f32, tag=ftag, name="fps")
                            for _ in range(2):
                                nc.tensor.matmul(
                                    fps[:], junk[:], junk[:],
                                    start=True, stop=True,
                                )
                    # folded even-u: contraction 1024, psum cols 256:512
                    for kq in range(QT // KQ):
                        xft = fin.tile([P, KQ, G], bf16, tag="xft", name="xft")
                        nc.gpsimd.dma_start(
                            xft[:],
                            xf_src[
                                kq * KQ * P:(kq + 1) * KQ * P,
                                g * G:(g + 1) * G,
                            ].rearrange("(o p) n -> p o n", p=P),
                        )
                        for ko in range(KQ):
                            jt = kq * KQ + ko
                            for jb in range(4):
                                nc.tensor.matmul(
                                    bk[jb][:, RO:G],
                                    xft[:, ko, jb * P:(jb + 1) * P],
                                    c0f_sb[:, jt, :],
                                    start=False,
                                    stop=(jt == QT - 1),
                                )
                    for jb in range(4):
                        nc.vector.tensor_copy(
                            t_sb[h][:, g * 4 + jb, :], bk[jb][:]
                        )
                if h == 0:
                    # column-fold butterflies on TE' (DVE work overlapping
                    # the b-half matmuls); partner tiles are partition-
                    # aligned thanks to the host column permutation.
                    for lvl, half in ((2, QT), (3, ET)):
                        for jt in range(half):
                            lo = t_sb[0][:, jt, :]
                            hi = t_sb[0][:, half + jt, :]
                            tmp = xin.tile(
                                [P, RB], bf16, tag="btmp", name="btmp",
                                bufs=2,
                            )
                            nc.vector.tensor_tensor(
                                tmp[:], lo, hi, mybir.AluOpType.subtract
                            )
                            nc.vector.tensor_tensor(
                                lo, lo, hi, mybir.AluOpType.add
                            )
                            nc.vector.tensor_copy(hi, tmp[:])

            # ---- stage 2 ----
            # v%8==0 / v%8==4: 512-deep contraction, one 512-col block each
            for sec in range(2):
                lhs_off = 0 if sec == 0 else ET
                bk = banks()
                ct = cin.tile([P, KQ, G], bf16, tag="ct", name="ct")
                nc.sync.dma_start(
                    ct[:],
                    c1v8_d[:, sec * G:(sec + 1) * G].rearrange(
                        "(o p) v -> p o v", p=P
                    ),
                )
                for jt in range(ET):
                    for mb in range(4):
                        nc.tensor.matmul(
                            bk[mb][:],
                            t_sb[0][:, lhs_off + jt, mb * P:(mb + 1) * P],
                            ct[:, jt, :],
                            start=(jt == 0),
                            stop=(jt == ET - 1),
                        )
                for mb in range(4):
                    ot = osb.tile([P, G], bf16, tag="ot", name="ot")
                    drain(bk[mb], mb, ot[:])
                    nc.gpsimd.dma_start(
                        out_d[mb * P:(mb + 1) * P, sec * G:(sec + 1) * G],
                        ot[:],
                    )
            # v%4==2: 1024-deep contraction over t2o (basis rows arrive
            # pre-reordered to match the permuted j'' layout)
            for blk in range(2):
                bk = banks()
                for jq in range(QT // KQ):
                    ct = cin.tile([P, KQ, G], bf16, tag="ct", name="ct")
                    nc.sync.dma_start(
                        ct[:],
                        c1v2_d[
                            jq * KQ * P:(jq + 1) * KQ * P,
                            blk * G:(blk + 1) * G,
                        ].rearrange("(o p) v -> p o v", p=P),
                    )
                    for jo in range(KQ):
                        jt = jq * KQ + jo
                        for mb in range(4):
                            nc.tensor.matmul(
                                bk[mb][:],
                                t_sb[0][:, QT + jt, mb * P:(mb + 1) * P],
                                ct[:, jo, :],
                                start=(jt == 0),
                                stop=(jt == QT - 1),
                            )
                for mb in range(4):
                    ot = osb.tile([P, G], bf16, tag="ot", name="ot")
                    drain(bk[mb], mb, ot[:])
                    nc.gpsimd.dma_start(
                        out_d[
                            mb * P:(mb + 1) * P,
                            Q + blk * G:Q + (blk + 1) * G,
                        ],
                        ot[:],
                    )
            # v odd: 2048-deep contraction over T(b)
            for vg in range(4):
                bk = banks()
                for jq in range(HT // KQ):
                    ct = cin.tile([P, KQ, G], bf16, tag="ct", name="ct")
                    nc.sync.dma_start(
                        ct[:],
                        c1vo_d[
                            jq * KQ * P:(jq + 1) * KQ * P,
                            vg * G:(vg + 1) * G,
                        ].rearrange("(o p) v -> p o v", p=P),
                    )
                    for jo in range(KQ):
                        jt = jq * KQ + jo
                        for mb in range(4):
                            nc.tensor.matmul(
                                bk[mb][:],
                                t_sb[1][:, jt, mb * P:(mb + 1) * P],
                                ct[:, jo, :],
                                start=(jt == 0),
                                stop=(jt == HT - 1),
                            )
                for mb in range(4):
                    ot = osb.tile([P, G], bf16, tag="ot", name="ot")
                    drain(bk[mb], mb, ot[:])
                    if vg == 3:
                        eng = (nc.sync, nc.scalar, nc.gpsimd, nc.scalar)[mb]
                    else:
                        eng = nc.gpsimd
                    eng.dma_start(
                        out_d[
                            mb * P:(mb + 1) * P,
                            2048 + vg * G:2048 + (vg + 1) * G,
                        ],
                        ot[:],
                    )
    nc.compile()
    return nc


def _get_nc():
    if "nc" not in _CACHE:
        _CACHE["nc"] = _build()
    return _CACHE["nc"]


def _dct_basis_t():
    """C^T as float32 [N, N]: C^T[i, k] = cos(pi*(2i+1)*k/(2N)).

    Matches the reference's float32 jnp computation (fp32 argument
    arithmetic) so basis rounding does not diverge from the oracle."""
    if "ct" in _CACHE:
        return _CACHE["ct"]
    ct = None
    try:
        import jax
        import jax.numpy as jnp

        cpus = jax.devices("cpu")
        with jax.default_device(cpus[0]):
            k = jnp.arange(N, dtype=jnp.float32)[:, None]
            i = jnp.arange(N, dtype=jnp.float32)[None, :]
            c = jnp.cos((jnp.pi / (2.0 * N)) * (2.0 * i + 1.0) * k)
            ct = np.ascontiguousarray(np.asarray(c).T)
    except Exception:
        pass
    if ct is None:
        k = np.arange(N, dtype=np.float32)[:, None]
        i = np.arange(N, dtype=np.float32)[None, :]
        s = math.pi / (2.0 * N)
        arg = (s * (2.0 * i + 1.0)).astype(np.float32) * k
        ct = np.ascontiguousarray(np.cos(arg.astype(np.float32)).T)
    _CACHE["ct"] = ct
    return ct


# column permutation: level-3-ready order inside each level-2 half
_IDX3 = np.concatenate([np.arange(E), np.arange(Q - 1, E - 1, -1)])
_PERM = np.concatenate([_IDX3, (H - 1) - _IDX3])


def _col_folds(xr):
    """a/b column halves of a row-variant: a = sum-fold (_PERM), b = diff."""
    import ml_dtypes

    bf = ml_dtypes.bfloat16
    a = np.ascontiguousarray((xr[:, :H] + xr[:, :H - 1:-1])[:, _PERM].astype(bf))
    b = np.ascontiguousarray((xr[:, :H] - xr[:, :H - 1:-1]).astype(bf))
    return a, b


def _in_maps(x):
    import ml_dtypes

    bf = ml_dtypes.bfloat16
    x = np.asarray(x, dtype=np.float32)
    ct = _dct_basis_t()

    # host row folds (exact fp32)
    xE = x[:H] + x[:H - 1:-1]
    xO = x[:H] - x[:H - 1:-1]
    E2d = xE[:Q] - xE[Q:][::-1]
    E2s = xE[:Q] + xE[Q:][::-1]
    E3s = E2s[:E] + E2s[E:][::-1]
    E3d = E2s[:E] - E2s[E:][::-1]

    xoa, xob = _col_folds(xO)
    f4a, f4b = _col_folds(E2d)  # %4-cores: E2d [1024, *]
    s3a, s3b = _col_folds(E3s)
    d3a, d3b = _col_folds(E3d)
    f8a = np.ascontiguousarray(np.concatenate([s3a, d3a], axis=0))
    f8b = np.ascontiguousarray(np.concatenate([s3b, d3b], axis=0))

    # stage-2 bases (shared)
    c1v8 = np.empty((E, Q), dtype=np.float32)
    c1v8[:, :G] = ct[:E, 0::8]
    c1v8[:, G:] = ct[:E, 4::8]
    c1v8 = c1v8.astype(bf)
    c1v2 = np.ascontiguousarray(ct[:Q, 2::4][_IDX3, :].astype(bf))
    c1vo = np.ascontiguousarray(ct[:H, 1::2].astype(bf))

    maps = []
    for c in range(NCORES):
        c0o = np.ascontiguousarray(
            ct[:H, 512 * c + 1:512 * (c + 1):2].astype(bf)
        )
        if c < 4:
            xfa, xfb = f4a, f4b
            c0f = np.ascontiguousarray(
                ct[:Q, 1024 * c + 2:1024 * c + 1024:4].astype(bf)
            )
        else:
            e = c - 4
            xfa, xfb = f8a, f8b
            c0f = np.zeros((Q, RO), dtype=bf)
            c0f[:E, :P] = ct[:E, 1024 * e:1024 * e + 1024:8].astype(bf)
            c0f[E:, P:] = ct[:E, 1024 * e + 4:1024 * e + 1024:8].astype(bf)
        maps.append(
            {
                "xoa": xoa,
                "xob": xob,
                "xfa": xfa,
                "xfb": xfb,
                "c0o": c0o,
                "c0f": c0f,
                "c1v8": c1v8,
                "c1v2": c1v2,
                "c1vo": c1vo,
            }
        )
    return maps


def _assemble(results):
    full = np.empty((N, N), dtype=np.float32)
    for c in range(NCORES):
        dev = np.asarray(results[c]["out"], dtype=np.float32)
        for rows, sl in (
            (dev[:RO], np.s_[512 * c + 1:512 * (c + 1):2]),
            (
                dev[RO:],
                np.s_[1024 * c + 2:1024 * c + 1024:4]
                if c < 4
                else None,
            ),
        ):
            if sl is not None:
                tgt = full[sl]
                tgt[:, 0::8] = rows[:, 0:512]
                tgt[:, 4::8] = rows[:, 512:1024]
                tgt[:, 2::4] = rows[:, 1024:2048]
                tgt[:, 1::2] = rows[:, 2048:4096]
            else:
                e = c - 4
                for half, sl8 in (
                    (rows[:P], np.s_[1024 * e:1024 * e + 1024:8]),
                    (rows[P:], np.s_[1024 * e + 4:1024 * e + 1024:8]),
                ):
                    tgt = full[sl8]
                    tgt[:, 0::8] = half[:, 0:512]
                    tgt[:, 4::8] = half[:, 512:1024]
                    tgt[:, 2::4] = half[:, 1024:2048]
                    tgt[:, 1::2] = half[:, 2048:4096]
    return full


def _run(x, **kwargs):
    nc = _get_nc()
    in_maps = _in_maps(x)
    last = None
    for attempt in range(3):
        try:
            res = run_bass_kernel_spmd(
                nc, in_maps, core_ids=list(range(NCORES)), **kwargs
            )
            return _assemble(res.results), res
        except Exception as e:  # transient NRT/device faults happen rarely
            last = e
    raise last


def kernel(x):
    out, _ = _run(x)
    return out


# revision 13
# speedup vs baseline: 1.0440x; 1.0440x over previous
"""2D DCT-II (4096x4096, fp32) on 8 TRN2 NeuronCores, bf16 matmuls.

out = C0 @ x @ C1^T with C0 = C1 = C, C[k, i] = cos(pi*(2i+1)*k/(2N)).

Fast-DCT folding via the basis reflection symmetries, applied on BOTH
axes. Column (v) axis: level-1 on the host (xa/xb halves), levels 2+3
on-device via DVE butterflies (unchanged from the fp32r baseline).
Row (u) axis: ALL levels on the host -- every core gets a balanced mix
of u-classes so no core is stuck with unfoldable work:

  core c (all 8):  256 odd-u rows   u = 512c+1..512c+511 step 2
                   -> contraction 2048 over xO (host level-1 diff)
  cores 0-3:       256 u%4==2 rows  u = 1024c+2 step 4
                   -> contraction 1024 over E2d = xE[:Q]-xE[Q:][::-1]
  cores 4-7 (e):   128 u%8==0 rows (u=1024e step 8)   over E3s [512]
                   128 u%8==4 rows (u=1024e+4 step 8) over E3d [512]

The folded-class contraction is UNIFIED across core types: xf [1024,*]
is E2d for %4-cores and [E3s; E3d] stacked for %8-cores, whose basis
c0f [1024, 256] is block-diagonal (zeros kill the cross terms). One
SPMD program, zero branching; zero extra DVE work (host row-folds are
free and replace the raw xE stream byte-for-byte).

Device pipeline per core (bf16 operands, fp32 PSUM):
  stage 1, per (half h, j'-group g): one PSUM bank per 128-j'-block;
    odd-u:   16 kt x moving-256 into cols 0:256   (lhsT = xo tiles)
    folded:   8 kt x moving-256 into cols 256:512 (lhsT = xf tiles)
    -> 768 matmuls total, intermediates land [j', m] in SBUF = the
    stationary layout stage 2 needs. m-packing = [odd 256 | folded 256].
  butterflies: t2e/t2o = TE'[:Q] -+ TE'[Q:];  t3e/t3o = t2e[:E] -+ t2e[E:]
  stage 2 (unchanged): v%8==0/4 (512-deep), v%4==2 (1024), v odd (2048)
    = 352 matmuls, all moving-512.
  Output leaves section-packed [v80|v84|v2|vodd] x [odd|folded] rows;
  the host scatters (pure numpy slicing).

PE work: 768*107ns + 352*213ns ~ 157us + warmup; HBM ~37 MB/core.
"""

import math

import numpy as np

import concourse.mybir as mybir
import concourse.tile as tile
from concourse import bacc
from concourse.bass_utils import run_bass_kernel_spmd

N = 4096
H = N // 2  # 2048: level-1 folded contraction
Q = N // 4  # 1024: level-2 folded contraction
E = N // 8  # 512:  level-3 folded contraction
P = 128
HT = H // P  # 16
QT = Q // P  # 8
ET = E // P  # 4
NCORES = 8
RB = 512  # output rows per core
RO = 256  # odd-u rows per core (psum cols 0:RO)
G = 512  # column-group / matmul moving width (stage 2)
KQ = 4  # k-tiles per streaming DMA

f32 = mybir.dt.float32
bf16 = mybir.dt.bfloat16

_CACHE = {}


def _build():
    nc = bacc.Bacc("TRN2", target_bir_lowering=False, debug=False)
    xoa_d = nc.dram_tensor("xoa", [4, 4, P, KQ, G], bf16, kind="ExternalInput")
    xob_d = nc.dram_tensor("xob", [4, 4, P, KQ, G], bf16, kind="ExternalInput")
    xfa_d = nc.dram_tensor("xfa", [2, 4, P, KQ, G], bf16, kind="ExternalInput")
    xfb_d = nc.dram_tensor("xfb", [2, 4, P, KQ, G], bf16, kind="ExternalInput")
    c0o_d = nc.dram_tensor("c0o", [P, HT, RO], bf16, kind="ExternalInput")
    c0f_d = nc.dram_tensor("c0f", [P, QT, RO], bf16, kind="ExternalInput")
    c1v8_d = nc.dram_tensor("c1v8", [2, P, KQ, G], bf16, kind="ExternalInput")
    c1v2_d = nc.dram_tensor("c1v2", [2, 2, P, KQ, G], bf16, kind="ExternalInput")
    c1vo_d = nc.dram_tensor("c1vo", [4, 4, P, KQ, G], bf16, kind="ExternalInput")
    out_d = nc.dram_tensor("out", [RB, N], bf16, kind="ExternalOutput")

    state = {"ggc": 0}

    with tile.TileContext(nc) as tc:
        with (
            tc.tile_pool(name="persist", bufs=1) as persist,
            tc.tile_pool(name="xin", bufs=6) as xin,
            tc.tile_pool(name="fin", bufs=3) as fin,
            tc.tile_pool(name="cin", bufs=6) as cin,
            tc.tile_pool(name="osb", bufs=3) as osb,
            tc.tile_pool(name="ps", bufs=1, space="PSUM") as ps,
        ):
            c0o_sb = persist.tile([P, HT, RO], bf16, tag="c0o", name="c0o_sb")
            c0f_sb = persist.tile([P, QT, RO], bf16, tag="c0f", name="c0f_sb")
            # intermediates [j', m] as [128, 16, 512]; m = [odd | folded]
            t_sb = [
                persist.tile([P, HT, RB], bf16, tag=f"t{h}", name=f"t{h}_sb")
                for h in range(2)
            ]

            def banks(n=4):
                g = state["ggc"]
                state["ggc"] += 1
                return [
                    ps.tile(
                        [P, G], f32, tag=f"ps{(g % 2) * 4 + i}",
                        name=f"ps{(g % 2) * 4 + i}",
                    )
                    for i in range(n)
                ]

            def drain(bk, mb, dst):
                # alternate DVE/ACT so section-end drains parallelize
                if mb % 2 == 0:
                    nc.vector.tensor_copy(dst, bk[:])
                else:
                    nc.scalar.copy(dst, bk[:])

            # PE warm-up: HAM clock gate needs ~3.4us of sustained matmul
            # activity; chew zeros while the first operands stream in.
            junk = persist.tile([P, P], f32, tag="junk", name="junk")
            nc.gpsimd.memset(junk[:], 0)
            jps = ps.tile([P, P], f32, tag="ps7", name="jps")
            for _ in range(28):
                nc.tensor.matmul(jps[:], junk[:], junk[:], start=True, stop=True)

            # ---- stage 1 ----
            # T(a|b)[j', m]: odd-u (16 kt over xo) + folded (8 kt over xf)
            for h in range(2):
                xo_src = xoa_d if h == 0 else xob_d
                xf_src = xfa_d if h == 0 else xfb_d
                for g in range(4):  # j'-column groups of 512
                    bk = banks()
                    # odd-u: contraction 2048, psum cols 0:256
                    for kq in range(HT // KQ):
                        if h == 0 and g == 0 and kq == 0:
                            # fine-grained first chunk: c0o k-tiles lead
                            # their x partners on the same (sync) queue so
                            # the first matmul starts after ~192 KB
                            for ko in range(KQ):
                                nc.sync.dma_start(
                                    c0o_sb[:, ko, :], c0o_d[:, ko, :]
                                )
                                if ko == 0:
                                    xt = xin.tile(
                                        [P, KQ, G], bf16, tag="xt", name="xt"
                                    )
                                nc.sync.dma_start(
                                    xt[:, ko, :], xo_src[0, 0, :, ko, :]
                                )
                            nc.scalar.dma_start(
                                c0o_sb[:, KQ:, :], c0o_d[:, KQ:, :]
                            )
                        else:
                            xt = xin.tile([P, KQ, G], bf16, tag="xt", name="xt")
                            nc.sync.dma_start(xt[:], xo_src[kq, g])
                        for ko in range(KQ):
                            it = kq * KQ + ko
                            for jb in range(4):
                                nc.tensor.matmul(
                                    bk[jb][:, 0:RO],
                                    xt[:, ko, jb * P:(jb + 1) * P],
                                    c0o_sb[:, it, :],
                                    start=(it == 0),
                                    stop=False,
                                )
                        if h == 0 and g == 0 and kq == 0:
                            # c0f for the folded class, early on gpsimd
                            nc.gpsimd.dma_start(c0f_sb[:], c0f_d[:])
                        if h == 0 and g < 2 and (g > 0 or kq >= 2):
                            # idle-bank warm fillers across the early
                            # HBM-starved chunk boundaries
                            ftag = "ps4" if g == 0 else "ps0"
                            fps = ps.tile([P, P], f32, tag=ftag, name="fps")
                            for _ in range(2):
                                nc.tensor.matmul(
                                    fps[:], junk[:], junk[:],
                                    start=True, stop=True,
                                )
                    # folded even-u: contraction 1024, psum cols 256:512
                    for kq in range(QT // KQ):
                        xft = fin.tile([P, KQ, G], bf16, tag="xft", name="xft")
                        nc.gpsimd.dma_start(xft[:], xf_src[kq, g])
                        for ko in range(KQ):
                            jt = kq * KQ + ko
                            for jb in range(4):
                                nc.tensor.matmul(
                                    bk[jb][:, RO:G],
                                    xft[:, ko, jb * P:(jb + 1) * P],
                                    c0f_sb[:, jt, :],
                                    start=False,
                                    stop=(jt == QT - 1),
                                )
                    for jb in range(4):
                        nc.vector.tensor_copy(
                            t_sb[h][:, g * 4 + jb, :], bk[jb][:]
                        )
                if h == 0:
                    # column-fold butterflies on TE' (DVE work overlapping
                    # the b-half matmuls); partner tiles are partition-
                    # aligned thanks to the host column permutation.
                    for lvl, half in ((2, QT), (3, ET)):
                        for jt in range(half):
                            lo = t_sb[0][:, jt, :]
                            hi = t_sb[0][:, half + jt, :]
                            tmp = xin.tile(
                                [P, RB], bf16, tag="btmp", name="btmp",
                                bufs=2,
                            )
                            nc.vector.tensor_tensor(
                                tmp[:], lo, hi, mybir.AluOpType.subtract
                            )
                            nc.vector.tensor_tensor(
                                lo, lo, hi, mybir.AluOpType.add
                            )
                            nc.vector.tensor_copy(hi, tmp[:])

            # ---- stage 2 ----
            # v%8==0 / v%8==4: 512-deep contraction, one 512-col block each
            for sec in range(2):
                lhs_off = 0 if sec == 0 else ET
                bk = banks()
                ct = cin.tile([P, KQ, G], bf16, tag="ct", name="ct")
                nc.sync.dma_start(ct[:], c1v8_d[sec])
                for jt in range(ET):
                    for mb in range(4):
                        nc.tensor.matmul(
                            bk[mb][:],
                            t_sb[0][:, lhs_off + jt, mb * P:(mb + 1) * P],
                            ct[:, jt, :],
                            start=(jt == 0),
                            stop=(jt == ET - 1),
                        )
                for mb in range(4):
                    ot = osb.tile([P, G], bf16, tag="ot", name="ot")
                    drain(bk[mb], mb, ot[:])
                    nc.gpsimd.dma_start(
                        out_d[mb * P:(mb + 1) * P, sec * G:(sec + 1) * G],
                        ot[:],
                    )
            # v%4==2: 1024-deep contraction over t2o (basis rows arrive
            # pre-reordered to match the permuted j'' layout)
            for blk in range(2):
                bk = banks()
                for jq in range(QT // KQ):
                    ct = cin.tile([P, KQ, G], bf16, tag="ct", name="ct")
                    nc.sync.dma_start(ct[:], c1v2_d[jq, blk])
                    for jo in range(KQ):
                        jt = jq * KQ + jo
                        for mb in range(4):
                            nc.tensor.matmul(
                                bk[mb][:],
                                t_sb[0][:, QT + jt, mb * P:(mb + 1) * P],
                                ct[:, jo, :],
                                start=(jt == 0),
                                stop=(jt == QT - 1),
                            )
                for mb in range(4):
                    ot = osb.tile([P, G], bf16, tag="ot", name="ot")
                    drain(bk[mb], mb, ot[:])
                    nc.gpsimd.dma_start(
                        out_d[
                            mb * P:(mb + 1) * P,
                            Q + blk * G:Q + (blk + 1) * G,
                        ],
                        ot[:],
                    )
            # v odd: 2048-deep contraction over T(b)
            for vg in range(4):
                bk = banks()
                for jq in range(HT // KQ):
                    ct = cin.tile([P, KQ, G], bf16, tag="ct", name="ct")
                    nc.sync.dma_start(ct[:], c1vo_d[jq, vg])
                    for jo in range(KQ):
                        jt = jq * KQ + jo
                        for mb in range(4):
                            nc.tensor.matmul(
                                bk[mb][:],
                                t_sb[1][:, jt, mb * P:(mb + 1) * P],
                                ct[:, jo, :],
                                start=(jt == 0),
                                stop=(jt == HT - 1),
                            )
                for mb in range(4):
                    ot = osb.tile([P, G], bf16, tag="ot", name="ot")
                    drain(bk[mb], mb, ot[:])
                    if vg == 3:
                        eng = (nc.sync, nc.scalar, nc.gpsimd, nc.scalar)[mb]
                    else:
                        eng = nc.gpsimd
                    eng.dma_start(
                        out_d[
                            mb * P:(mb + 1) * P,
                            2048 + vg * G:2048 + (vg + 1) * G,
                        ],
                        ot[:],
                    )
    nc.compile()
    return nc


def _get_nc():
    if "nc" not in _CACHE:
        _CACHE["nc"] = _build()
    return _CACHE["nc"]


def _dct_basis_t():
    """C^T as float32 [N, N]: C^T[i, k] = cos(pi*(2i+1)*k/(2N)).

    Matches the reference's float32 jnp computation (fp32 argument
    arithmetic) so basis rounding does not diverge from the oracle."""
    if "ct" in _CACHE:
        return _CACHE["ct"]
    ct = None
    try:
        import jax
        import jax.numpy as jnp

        cpus = jax.devices("cpu")
        with jax.default_device(cpus[0]):
            k = jnp.arange(N, dtype=jnp.float32)[:, None]
            i = jnp.arange(N, dtype=jnp.float32)[None, :]
            c = jnp.cos((jnp.pi / (2.0 * N)) * (2.0 * i + 1.0) * k)
            ct = np.ascontiguousarray(np.asarray(c).T)
    except Exception:
        pass
    if ct is None:
        k = np.arange(N, dtype=np.float32)[:, None]
        i = np.arange(N, dtype=np.float32)[None, :]
        s = math.pi / (2.0 * N)
        arg = (s * (2.0 * i + 1.0)).astype(np.float32) * k
        ct = np.ascontiguousarray(np.cos(arg.astype(np.float32)).T)
    _CACHE["ct"] = ct
    return ct


# column permutation: level-3-ready order inside each level-2 half
_IDX3 = np.concatenate([np.arange(E), np.arange(Q - 1, E - 1, -1)])
_PERM = np.concatenate([_IDX3, (H - 1) - _IDX3])


def _tile_x(x2d):
    """[R, C] -> [R/512, C/512, 128, 4, 512]: (kq, g, p, o, n) pre-tiled so
    each streamed chunk is one fully-contiguous DMA (4 KB/partition)."""
    R, C = x2d.shape
    return np.ascontiguousarray(
        x2d.reshape(R // 512, KQ, P, C // G, G).transpose(0, 3, 2, 1, 4)
    )


def _tile_c(c2d):
    """[R, M] -> [128, R/128, M] partition-major pre-tiled."""
    R, M = c2d.shape
    return np.ascontiguousarray(
        c2d.reshape(R // P, P, M).transpose(1, 0, 2)
    )


def _col_folds(xr):
    """a/b column halves of a row-variant: a = sum-fold (_PERM), b = diff,
    both pre-tiled for streaming."""
    import ml_dtypes

    bf = ml_dtypes.bfloat16
    a = _tile_x((xr[:, :H] + xr[:, :H - 1:-1])[:, _PERM].astype(bf))
    b = _tile_x((xr[:, :H] - xr[:, :H - 1:-1]).astype(bf))
    return a, b


def _in_maps(x):
    import ml_dtypes

    bf = ml_dtypes.bfloat16
    x = np.asarray(x, dtype=np.float32)
    ct = _dct_basis_t()

    # host row folds (exact fp32)
    xE = x[:H] + x[:H - 1:-1]
    xO = x[:H] - x[:H - 1:-1]
    E2d = xE[:Q] - xE[Q:][::-1]
    E2s = xE[:Q] + xE[Q:][::-1]
    E3s = E2s[:E] + E2s[E:][::-1]
    E3d = E2s[:E] - E2s[E:][::-1]

    xoa, xob = _col_folds(xO)
    f4a, f4b = _col_folds(E2d)  # %4-cores: E2d [1024, *]
    s3a, s3b = _col_folds(E3s)
    d3a, d3b = _col_folds(E3d)
    # [E3s; E3d] stacked along the contraction: kq0 = E3s, kq1 = E3d
    f8a = np.ascontiguousarray(np.concatenate([s3a, d3a], axis=0))
    f8b = np.ascontiguousarray(np.concatenate([s3b, d3b], axis=0))

    # stage-2 bases (shared), pre-tiled
    c1v8 = np.empty((E, Q), dtype=np.float32)
    c1v8[:, :G] = ct[:E, 0::8]
    c1v8[:, G:] = ct[:E, 4::8]
    c1v8 = np.ascontiguousarray(_tile_x(c1v8.astype(bf))[0])  # (sec,p,o,n)
    c1v2 = _tile_x(ct[:Q, 2::4][_IDX3, :].astype(bf))  # (jq, blk, p, o, n)
    c1vo = _tile_x(ct[:H, 1::2].astype(bf))  # (jq, vg, p, o, n)

    maps = []
    for c in range(NCORES):
        c0o = _tile_c(ct[:H, 512 * c + 1:512 * (c + 1):2].astype(bf))
        if c < 4:
            xfa, xfb = f4a, f4b
            c0f = _tile_c(
                ct[:Q, 1024 * c + 2:1024 * c + 1024:4].astype(bf)
            )
        else:
            e = c - 4
            xfa, xfb = f8a, f8b
            c0f = np.zeros((Q, RO), dtype=bf)
            c0f[:E, :P] = ct[:E, 1024 * e:1024 * e + 1024:8].astype(bf)
            c0f[E:, P:] = ct[:E, 1024 * e + 4:1024 * e + 1024:8].astype(bf)
            c0f = _tile_c(c0f)
        maps.append(
            {
                "xoa": xoa,
                "xob": xob,
                "xfa": xfa,
                "xfb": xfb,
                "c0o": c0o,
                "c0f": c0f,
                "c1v8": c1v8,
                "c1v2": c1v2,
                "c1vo": c1vo,
            }
        )
    return maps


def _assemble(results):
    full = np.empty((N, N), dtype=np.float32)
    for c in range(NCORES):
        dev = np.asarray(results[c]["out"], dtype=np.float32)
        for rows, sl in (
            (dev[:RO], np.s_[512 * c + 1:512 * (c + 1):2]),
            (
                dev[RO:],
                np.s_[1024 * c + 2:1024 * c + 1024:4]
                if c < 4
                else None,
            ),
        ):
            if sl is not None:
                tgt = full[sl]
                tgt[:, 0::8] = rows[:, 0:512]
                tgt[:, 4::8] = rows[:, 512:1024]
                tgt[:, 2::4] = rows[:, 1024:2048]
                tgt[:, 1::2] = rows[:, 2048:4096]
            else:
                e = c - 4
                for half, sl8 in (
                    (rows[:P], np.s_[1024 * e:1024 * e + 1024:8]),
                    (rows[P:], np.s_[1024 * e + 4:1024 * e + 1024:8]),
                ):
                    tgt = full[sl8]
                    tgt[:, 0::8] = half[:, 0:512]
                    tgt[:, 4::8] = half[:, 512:1024]
                    tgt[:, 2::4] = half[:, 1024:2048]
                    tgt[:, 1::2] = half[:, 2048:4096]
    return full


def _run(x, **kwargs):
    nc = _get_nc()
    in_maps = _in_maps(x)
    last = None
    for attempt in range(3):
        try:
            res = run_bass_kernel_spmd(
                nc, in_maps, core_ids=list(range(NCORES)), **kwargs
            )
            return _assemble(res.results), res
        except Exception as e:  # transient NRT/device faults happen rarely
            last = e
    raise last


def kernel(x):
    out, _ = _run(x)
    return out
